# revision 20
# baseline (speedup 1.0000x reference)
"""MoE layer kernel for 8x TRN2 NeuronCores (Bass/Tile).

Math (reference):
    w      = softmax(x @ gate_W + gate_b, axis=-1)[:E]          # [E, F]
    W_eff  = einsum('ef,edf->df', w, expert_W)                  # [D, F]
    b_eff  = einsum('ef,ef->f',  w, expert_b)                   # [F]
    out    = x @ W_eff + b_eff                                  # [N, F]

Sharding: F-parallel across 8 cores (each core owns 128 f-columns).
  - gate_W/gate_b are column-rolled per core so the shard is columns 0:128
    (the softmax row-sum is order invariant, so rolling columns is harmless).
  - x, gate_W, gate_b, expert_W, expert_b all cast fp16 on the host
    (tolerance 2e-2 >> fp16's ~5e-4; halves the HBM stream, which is the
    roofline). expert_W shard kept in natural [e, d, f] order — no host
    transpose.
  - Each core computes out[:, shard].T as fp16 [128, 4096]; the host
    casts back and concatenates.

Device algorithm per core:
  1. Load x in [128, 256] fp16 chunks, build xT [2][128, 4096] fp16 via
     PE transposes.
  2. Gate GEMM (tokens 0..1023 == experts 0..1023) + bias + exp (+row-sum
     via accum_out) + normalize -> wnorm16 [e_p, et, f] fp16. wnorm's
     partition axis IS the expert index (tokens 0..1023), blocked by
     et = e // 128 — exactly the stationary layout the PE trick needs.
  3. W_eff shard via the PE "diagonal" trick: for each d,
     psum[f, f'] = sum_e wnorm[e, f] * ew[e, d, f]   (8 e-tile matmuls
     accumulated in PSUM; moving tile covers 4 d's -> psum [128, 512]).
     The wanted values sit on the diagonal f == f'; a DVE
     scalar_tensor_tensor(psum x identity, accum_out) extracts each d's
     diagonal into wefft[:, d]. PE ~109us and DVE ~92us both hide under
     the fp16 DMA stream (~190us), the per-core HBM roofline (358 GB/s).
  4. b_eff via the same diag trick against expert_b (one matmul group).
  5. Transpose wefft -> W_eff [d_p, f] fp16; out^T = W_eff^T @ x^T on PE
     (fp16); bias fused into the ACT psum->SBUF copy (per-partition
     bias); chunked DMA out (fp16).

NOTE: this walrus build rejects any instruction carrying more than ONE
semaphore wait ("Too many sync wait commands"). _split_multi_waits()
post-processes the scheduled program, hoisting extra waits onto standalone
EventSemaphore instructions on the same engine queue (the same primitive
Tile's own barriers use).
"""

import numpy as np

N, D, E, F = 4096, 256, 1024, 1024
NCORES = 8
FSH = F // NCORES  # 128 f-columns per core
P = 128

_CACHE = {}
LAST_RESULT = None


def _split_multi_waits(nc):
    """Split multi-wait instructions into chains of single-wait ones."""
    import concourse.mybir as mybir

    n = 0
    for fn in nc.m.functions:
        for bb in fn.blocks:
            out = []
            changed = False
            for ins in bb.instructions:
                si = ins.sync_info
                if si is not None and si.on_wait and len(si.on_wait) > 1:
                    waits = list(si.on_wait)
                    for w in waits[:-1]:
                        es = mybir.InstEventSemaphore(
                            name=f"wsplit_{n}",
                            engine=ins.engine,
                            sync_info=mybir.SyncInfo(
                                on_wait=[w], on_update=[]),
                        )
                        out.append(es)
                        n += 1
                    ins.sync_info = mybir.SyncInfo(
                        on_wait=[waits[-1]], on_update=list(si.on_update))
                    changed = True
                out.append(ins)
            if changed:
                bb.instructions = out
    return n


def _build_bass(repeats=1, dsg_size=16, prefetch_dsgs=3):
    import concourse.bass as bass
    import concourse.mybir as mybir
    from concourse.masks import make_identity
    from concourse.tile import TileContext

    f32 = mybir.dt.float32
    f16 = mybir.dt.float16
    AF = mybir.ActivationFunctionType
    mult = mybir.AluOpType.mult

    nc = bass.Bass(trn_type="TRN2", name="moe_fshard",
                   dynamic_dma_scratch_size=4096)

    x_d = nc.dram_tensor("x", [N, D], f16, kind="ExternalInput")
    gw_d = nc.dram_tensor("gw", [D, F], f16, kind="ExternalInput")
    gb_d = nc.dram_tensor("gb", [1, F], f16, kind="ExternalInput")
    # expert_W shard [E, D, FSH] fp16, natural order
    ew_d = nc.dram_tensor("ewt", [E, D, FSH], f16, kind="ExternalInput")
    # expert_b shard [E, FSH] fp16, natural order
    eb_d = nc.dram_tensor("ebt", [E, FSH], f16, kind="ExternalInput")
    out_d = nc.dram_tensor("outT", [FSH, N], f16, kind="ExternalOutput")

    EBLK = E // P       # 8 e-tiles (gate token blocks)
    TTILE = N // P      # 32 x chunks
    DH = D // P         # 2 halves of d
    NDSG = D // dsg_size            # d super-groups
    NBANK = dsg_size * P // 512     # psum banks per super-group (4 d each)

    with TileContext(nc) as tc:
        with tc.tile_pool(name="persist", bufs=1) as persist, \
             tc.tile_pool(name="xcp", bufs=8) as xcp, \
             tc.tile_pool(name="wep",
                          bufs=(1 + prefetch_dsgs) * EBLK) as wep:

            # smalls packs tiny constants:
            #  [:,0:128] identity f32; [:,257:258] beff_col;
            #  row0 258:259 act_scratch
            smalls = persist.tile([P, 512], f32)
            ident = smalls[:, 0:128]
            beff_col = smalls[:, 257:258]
            act_scr = smalls[0:1, 258:259]

            sm16 = persist.tile([P, 256], f16)
            ident16 = sm16[:, 0:128]
            ones16_r = sm16[0:1, 128:256]

            xT = persist.tile([P, DH, N], f16)          # 16KB/part
            wnorm16 = persist.tile([P, EBLK, FSH], f16)  # 2KB/part
            wefft = persist.tile([P, D], f32)           # 1KB/part
            weff = persist.tile([P, DH * FSH], f16)     # 0.5KB/part
            scr = persist.tile([P, 2], f32)             # rsum, rcp
            rsum = scr[:, 0:1]
            rcp = scr[:, 1:2]
            expsc = persist.tile([P, F], f32)           # 4KB/part
            junk = persist.tile([P, P], f32)            # extract junk out
            outT_sb = persist.tile([P, N], f16)         # 8KB/part
            gw_sb = persist.tile([P, DH, F], f16)       # 4KB/part
            gb_sb = persist.tile([1, F], f16)
            ebT_sb = persist.tile([P, EBLK, FSH], f16)  # 2KB/part

            # ---- small input DMAs first on the Sync ring (needed early)
            nc.sync.dma_start(
                out=gw_sb[:], in_=gw_d.rearrange("(h p) f -> p h f", p=P))
            nc.sync.dma_start(out=gb_sb[:], in_=gb_d[:, :])
            nc.sync.dma_start(
                out=ebT_sb[:], in_=eb_d.rearrange("(t p) f -> p t f", p=P))

            for rep in range(repeats):
                # -- expert-W stream: tile (dsg, et) = [128e, dsg_size d,
                # 128 f] fp16, issued dsg-granular, prefetch_dsgs ahead,
                # alternating the Sync/Scalar HWDGE rings.
                wet_tiles = {}

                def issue_dsg(dsg):
                    tiles = []
                    for et in range(EBLK):
                        w = wep.tile([P, dsg_size, FSH], f16, tag="we",
                                     name=f"wet{rep}_{dsg}_{et}")
                        eng = nc.sync if (dsg * EBLK + et) % 2 == 0 \
                            else nc.scalar
                        eng.dma_start(
                            out=w[:],
                            in_=ew_d[et * P:(et + 1) * P,
                                     dsg * dsg_size:(dsg + 1) * dsg_size,
                                     :])
                        tiles.append(w)
                    wet_tiles[dsg] = tiles

                # x-chunk DMAs ride the (mostly idle) Vector ring
                xcs = {}

                def xchunk_dma(a):
                    xc = xcp.tile([P, D], f16, tag="xc",
                                  name=f"xc{rep}_{a}")
                    nc.sync.dma_start(
                        out=xc[:], in_=x_d[a * P:(a + 1) * P, :])
                    xcs[a] = xc

                def xchunk_transpose(a, pool, tag):
                    xc = xcs.pop(a)
                    for dh in range(DH):
                        pt = pool.tile([P, P], f16, tag=tag, bufs=2,
                                       name=f"pt{rep}_{a}_{dh}")
                        nc.tensor.transpose(
                            pt[:], xc[:, dh * P:(dh + 1) * P], ident16)
                        nc.scalar.copy(
                            xT[:, dh, a * P:(a + 1) * P], pt[:])

                for dsg in range(prefetch_dsgs):
                    issue_dsg(dsg)
                for a in range(EBLK):
                    xchunk_dma(a)

                # constants (gpsimd iota + copies), after the DMA issues
                make_identity(nc, ident)
                nc.scalar.copy(ident16[:, :], ident)
                nc.vector.memset(sm16[:, 128:256], 1.0)

                # map extra x chunks (8..31) onto phase-2 dsg slots
                xtra = list(range(EBLK, TTILE))
                tdsg = {c: (i * NDSG) // len(xtra)
                        for i, c in enumerate(xtra)}
                dma_at = {}
                trans_at = {}
                for c in xtra:
                    dma_at.setdefault(max(0, tdsg[c] - 3), []).append(c)
                    trans_at.setdefault(tdsg[c], []).append(c)

                # ============ Phase 1: gate + softmax ==================
                with tc.tile_pool(name=f"dummyp{rep}", bufs=1,
                                  space="PSUM") as dummyp, \
                     tc.tile_pool(name=f"tpsum{rep}", bufs=2,
                                  space="PSUM") as tpsum, \
                     tc.tile_pool(name=f"gpsum{rep}", bufs=2,
                                  space="PSUM") as gpsum:

                    dummy = dummyp.tile([1, 1], f32)
                    # PE touch: absorb gpsimd tick (identity)
                    nc.tensor.matmul(dummy[:], ident16[:, 0:1],
                                     ident16[:, 0:1],
                                     start=True, stop=True)

                    for a in range(EBLK):
                        xchunk_transpose(a, tpsum, "pt")

                    for a in range(EBLK):
                        lp = gpsum.tile([P, F], f32, tag="lp",
                                        name=f"lp{rep}_{a}")
                        for half in range(2):
                            sl = slice(half * 512, (half + 1) * 512)
                            nc.tensor.matmul(lp[:, sl],
                                             xT[:, 0, a * P:(a + 1) * P],
                                             gw_sb[:, 0, sl],
                                             start=True, stop=False)
                            nc.tensor.matmul(lp[:, sl],
                                             xT[:, 1, a * P:(a + 1) * P],
                                             gw_sb[:, 1, sl],
                                             start=False, stop=False)
                            nc.tensor.matmul(lp[:, sl], ones16_r,
                                             gb_sb[0:1, sl],
                                             start=False, stop=True)
                        if a >= 1:
                            # ACT touch: absorb ts_mul(a-1)'s DVE tick
                            nc.scalar.copy(act_scr, wnorm16[0:1, a - 1, 0:1])
                        nc.scalar.activation(expsc[:], lp[:], AF.Exp,
                                             accum_out=rsum)
                        nc.vector.reciprocal(rcp, rsum)
                        nc.vector.tensor_scalar_mul(
                            wnorm16[:, a, :], expsc[:, 0:FSH], rcp)

                    # ====== b_eff via PE diag trick ====================
                    bp = tpsum.tile([P, P], f32, tag="bpt", bufs=1,
                                    name=f"bp{rep}")
                    for et in range(EBLK):
                        nc.tensor.matmul(bp[:], wnorm16[:, et, :],
                                         ebT_sb[:, et, :],
                                         start=(et == 0),
                                         stop=(et == EBLK - 1))
                    nc.vector.scalar_tensor_tensor(
                        out=junk[:], in0=bp[:], scalar=1.0,
                        in1=ident, op0=mult, op1=mult,
                        accum_out=beff_col)

                # == Phase 2: W_eff^T diag-matmul on PE, extract on DVE ==
                # x-chunk transposes for chunks 8..31 ride along here
                # (PE is ~40% idle while the DMA stream paces the phase).
                with tc.tile_pool(name=f"wpsum{rep}", bufs=6,
                                  space="PSUM") as wpsum:
                    for dsg in range(NDSG):
                        if dsg + prefetch_dsgs < NDSG:
                            issue_dsg(dsg + prefetch_dsgs)
                        for c in dma_at.get(dsg, []):
                            xchunk_dma(c)
                        for c in trans_at.get(dsg, []):
                            xchunk_transpose(c, wpsum, "pt2")
                        banks = []
                        for b in range(NBANK):
                            bk = wpsum.tile([P, 512], f32, tag="wp",
                                            name=f"wp{rep}_{dsg}_{b}")
                            banks.append(bk)
                        wets = wet_tiles.pop(dsg)
                        for b in range(NBANK):
                            for et in range(EBLK):
                                nc.tensor.matmul(
                                    banks[b][:],
                                    wnorm16[:, et, :],
                                    wets[et][:, b * 4:(b + 1) * 4, :],
                                    start=(et == 0),
                                    stop=(et == EBLK - 1))
                        for b in range(NBANK):
                            for di in range(4):
                                d = dsg * dsg_size + b * 4 + di
                                nc.vector.scalar_tensor_tensor(
                                    out=junk[:],
                                    in0=banks[b][:, di * P:(di + 1) * P],
                                    scalar=1.0, in1=ident,
                                    op0=mult, op1=mult,
                                    accum_out=wefft[:, d:d + 1])

                # ====== Phase 3: W_eff transpose + final GEMM ==========
                with tc.tile_pool(name=f"fpsum{rep}", bufs=2,
                                  space="PSUM") as fpsum:
                    for dh in range(DH):
                        pt3 = fpsum.tile([P, P], f32, tag="pt3",
                                         name=f"pt3{rep}_{dh}")
                        nc.tensor.transpose(
                            pt3[:], wefft[:, dh * P:(dh + 1) * P], ident)
                        nc.scalar.copy(
                            weff[:, dh * FSH:(dh + 1) * FSH], pt3[:])
                    for ch in range(N // 512):
                        sl = slice(ch * 512, (ch + 1) * 512)
                        ps = fpsum.tile([P, 512], f32, tag="fp",
                                        name=f"fp{rep}_{ch}")
                        nc.tensor.matmul(ps[:], weff[:, 0:FSH],
                                         xT[:, 0, sl],
                                         start=True, stop=False)
                        nc.tensor.matmul(ps[:], weff[:, FSH:2 * FSH],
                                         xT[:, 1, sl],
                                         start=False, stop=True)
                        # psum->SBUF copy with per-partition bias add
                        nc.scalar.activation(outT_sb[:, sl], ps[:],
                                             AF.Identity, bias=beff_col,
                                             scale=1.0)
                        nc.sync.dma_start(out=out_d[:, sl],
                                          in_=outT_sb[:, sl])

    _split_multi_waits(nc)
    return nc


def _prep_in_maps(x, gate_W, gate_b, expert_W, expert_b):
    x16 = np.ascontiguousarray(np.asarray(x).astype(np.float16))
    gate_W = np.asarray(gate_W, dtype=np.float32)
    gate_b = np.asarray(gate_b, dtype=np.float32).reshape(1, F)
    expert_W = np.asarray(expert_W, dtype=np.float32)
    expert_b = np.asarray(expert_b, dtype=np.float32)

    in_maps = []
    for c in range(NCORES):
        sh = slice(c * FSH, (c + 1) * FSH)
        in_maps.append({
            "x": x16,
            # roll shard columns to the front; softmax row-sum is invariant
            "gw": np.ascontiguousarray(
                np.roll(gate_W, -c * FSH, axis=1).astype(np.float16)),
            "gb": np.ascontiguousarray(
                np.roll(gate_b, -c * FSH, axis=1).astype(np.float16)),
            # natural [e, d, f] / [e, f] order, fp16
            "ewt": np.ascontiguousarray(
                expert_W[:, :, sh].astype(np.float16)),
            "ebt": np.ascontiguousarray(
                expert_b[:, sh].astype(np.float16)),
        })
    return in_maps


def kernel(x, gate_W, gate_b, expert_W, expert_b, _trace=False):
    global LAST_RESULT
    from concourse.bass_utils import run_bass_kernel_spmd

    if "nc" not in _CACHE:
        _CACHE["nc"] = _build_bass()
    nc = _CACHE["nc"]

    in_maps = _prep_in_maps(x, gate_W, gate_b, expert_W, expert_b)

    res = run_bass_kernel_spmd(
        nc, in_maps, core_ids=list(range(NCORES)), trace=_trace,
    )
    LAST_RESULT = res

    out = np.empty([N, F], dtype=np.float32)
    for c in range(NCORES):
        out[:, c * FSH:(c + 1) * FSH] = \
            res.results[c]["outT"].astype(np.float32).T
    return out


# revision 23
# speedup vs baseline: 1.1183x; 1.1183x over previous
"""MoE layer kernel for 8x TRN2 NeuronCores (Bass/Tile).

Math (reference):
    w      = softmax(x @ gate_W + gate_b, axis=-1)[:E]          # [E, F]
    W_eff  = einsum('ef,edf->df', w, expert_W)                  # [D, F]
    b_eff  = einsum('ef,ef->f',  w, expert_b)                   # [F]
    out    = x @ W_eff + b_eff                                  # [N, F]

Sharding: F-parallel across 8 cores (each core owns 128 f-columns).
  - gate_W/gate_b are column-rolled per core so the shard is columns 0:128
    (the softmax row-sum is order invariant, so rolling columns is harmless).
  - x, gate_W, gate_b, expert_W, expert_b all cast fp16 on the host
    (tolerance 2e-2 >> fp16's ~5e-4; halves the HBM stream, which is the
    roofline). expert_W shard kept in natural [e, d, f] order — no host
    transpose.
  - Each core computes out[:, shard].T as fp16 [128, 4096]; the host
    casts back and concatenates.

Device algorithm per core:
  1. Load x in [128, 256] fp16 chunks, build xT [2][128, 4096] fp16 via
     PE transposes.
  2. Gate GEMM (tokens 0..1023 == experts 0..1023) + bias + exp (+row-sum
     via accum_out) + normalize -> wnorm16 [e_p, et, f] fp16. wnorm's
     partition axis IS the expert index (tokens 0..1023), blocked by
     et = e // 128 — exactly the stationary layout the PE trick needs.
  3. W_eff shard via the PE "diagonal" trick: for each d,
     psum[f, f'] = sum_e wnorm[e, f] * ew[e, d, f]   (8 e-tile matmuls
     accumulated in PSUM; moving tile covers 4 d's -> psum [128, 512]).
     The wanted values sit on the diagonal f == f'; a DVE
     scalar_tensor_tensor(psum x identity, accum_out) extracts each d's
     diagonal into wefft[:, d]. PE ~109us and DVE ~92us both hide under
     the fp16 DMA stream (~190us), the per-core HBM roofline (358 GB/s).
  4. b_eff via the same diag trick against expert_b (one matmul group).
  5. Transpose wefft -> W_eff [d_p, f] fp16; out^T = W_eff^T @ x^T on PE
     (fp16); bias fused into the ACT psum->SBUF copy (per-partition
     bias); chunked DMA out (fp16).

NOTE: this walrus build rejects any instruction carrying more than ONE
semaphore wait ("Too many sync wait commands"). _split_multi_waits()
post-processes the scheduled program, hoisting extra waits onto standalone
EventSemaphore instructions on the same engine queue (the same primitive
Tile's own barriers use).
"""

import numpy as np

N, D, E, F = 4096, 256, 1024, 1024
NCORES = 8
FSH = F // NCORES  # 128 f-columns per core
P = 128

_CACHE = {}
LAST_RESULT = None


def _split_multi_waits(nc):
    """Split multi-wait instructions into chains of single-wait ones."""
    import concourse.mybir as mybir

    n = 0
    for fn in nc.m.functions:
        for bb in fn.blocks:
            out = []
            changed = False
            for ins in bb.instructions:
                si = ins.sync_info
                if si is not None and si.on_wait and len(si.on_wait) > 1:
                    waits = list(si.on_wait)
                    for w in waits[:-1]:
                        es = mybir.InstEventSemaphore(
                            name=f"wsplit_{n}",
                            engine=ins.engine,
                            sync_info=mybir.SyncInfo(
                                on_wait=[w], on_update=[]),
                        )
                        out.append(es)
                        n += 1
                    ins.sync_info = mybir.SyncInfo(
                        on_wait=[waits[-1]], on_update=list(si.on_update))
                    changed = True
                out.append(ins)
            if changed:
                bb.instructions = out
    return n


def _build_bass(repeats=1, dsg_size=16, prefetch_dsgs=3):
    import concourse.bass as bass
    import concourse.mybir as mybir
    from concourse.masks import make_identity
    from concourse.tile import TileContext

    f32 = mybir.dt.float32
    f16 = mybir.dt.float16
    AF = mybir.ActivationFunctionType
    mult = mybir.AluOpType.mult

    nc = bass.Bass(trn_type="TRN2", name="moe_fshard",
                   dynamic_dma_scratch_size=4096)

    x_d = nc.dram_tensor("x", [N, D], f16, kind="ExternalInput")
    gw_d = nc.dram_tensor("gw", [D, F], f16, kind="ExternalInput")
    gb_d = nc.dram_tensor("gb", [1, F], f16, kind="ExternalInput")
    # expert_W shard [E, D, FSH] fp16, natural order
    ew_d = nc.dram_tensor("ewt", [E, D, FSH], f16, kind="ExternalInput")
    # expert_b shard [E, FSH] fp16, natural order
    eb_d = nc.dram_tensor("ebt", [E, FSH], f16, kind="ExternalInput")
    out_d = nc.dram_tensor("outT", [FSH, N], f16, kind="ExternalOutput")

    EBLK = E // P       # 8 e-tiles (gate token blocks)
    TTILE = N // P      # 32 x chunks
    DH = D // P         # 2 halves of d
    NDSG = D // dsg_size            # d super-groups
    NBANK = dsg_size * P // 512     # psum banks per super-group (4 d each)

    with TileContext(nc) as tc:
        with tc.tile_pool(name="persist", bufs=1) as persist, \
             tc.tile_pool(name="xcp", bufs=8) as xcp, \
             tc.tile_pool(name="wep",
                          bufs=(1 + prefetch_dsgs) * EBLK) as wep:

            # smalls packs tiny constants:
            #  [:,0:128] identity f32; [:,257:258] beff_col;
            #  row0 258:259 act_scratch
            smalls = persist.tile([P, 512], f32)
            ident = smalls[:, 0:128]
            beff_col = smalls[:, 257:258]
            act_scr = smalls[0:1, 258:259]

            sm16 = persist.tile([P, 256], f16)
            ident16 = sm16[:, 0:128]
            ones16_r = sm16[0:1, 128:256]

            xT = persist.tile([P, DH, N], f16)          # 16KB/part
            wnorm16 = persist.tile([P, EBLK, FSH], f16)  # 2KB/part
            wefft = persist.tile([P, D], f32)           # 1KB/part
            weff = persist.tile([P, DH * FSH], f16)     # 0.5KB/part
            scr = persist.tile([P, 2], f32)             # rsum, rcp
            rsum = scr[:, 0:1]
            rcp = scr[:, 1:2]
            expsc = persist.tile([P, F], f32)           # 4KB/part
            junk = persist.tile([P, P], f32)            # extract junk out
            outT_sb = persist.tile([P, N], f16)         # 8KB/part
            gw_sb = persist.tile([P, DH, F], f16)       # 4KB/part
            gb_sb = persist.tile([1, F], f16)
            ebT_sb = persist.tile([P, EBLK, FSH], f16)  # 2KB/part

            # ---- small input DMAs first on the Sync ring (needed early)
            nc.sync.dma_start(
                out=gw_sb[:], in_=gw_d.rearrange("(h p) f -> p h f", p=P))
            nc.sync.dma_start(out=gb_sb[:], in_=gb_d[:, :])
            nc.sync.dma_start(
                out=ebT_sb[:], in_=eb_d.rearrange("(t p) f -> p t f", p=P))

            for rep in range(repeats):
                # -- expert-W stream: tile (dsg, et) = [128e, dsg_size d,
                # 128 f] fp16, issued dsg-granular, prefetch_dsgs ahead,
                # alternating the Sync/Scalar HWDGE rings.
                wet_tiles = {}

                def issue_dsg(dsg):
                    tiles = []
                    for et in range(EBLK):
                        w = wep.tile([P, dsg_size, FSH], f16, tag="we",
                                     name=f"wet{rep}_{dsg}_{et}")
                        eng = nc.sync if (dsg * EBLK + et) % 2 == 0 \
                            else nc.scalar
                        eng.dma_start(
                            out=w[:],
                            in_=ew_d[et * P:(et + 1) * P,
                                     dsg * dsg_size:(dsg + 1) * dsg_size,
                                     :])
                        tiles.append(w)
                    wet_tiles[dsg] = tiles

                # x-chunk DMAs ride the (mostly idle) Vector ring
                xcs = {}

                def xchunk_dma(a):
                    xc = xcp.tile([P, D], f16, tag="xc",
                                  name=f"xc{rep}_{a}")
                    # phase-1 chunks ride the Scalar ring ahead of the
                    # ew-odds; later chunks alternate to spread issue cost
                    eng = nc.scalar if a % 2 == 0 else nc.sync
                    eng.dma_start(
                        out=xc[:], in_=x_d[a * P:(a + 1) * P, :])
                    xcs[a] = xc

                def xchunk_transpose(a, pool, tag):
                    xc = xcs.pop(a)
                    for dh in range(DH):
                        pt = pool.tile([P, P], f16, tag=tag, bufs=2,
                                       name=f"pt{rep}_{a}_{dh}")
                        nc.tensor.transpose(
                            pt[:], xc[:, dh * P:(dh + 1) * P], ident16)
                        nc.scalar.copy(
                            xT[:, dh, a * P:(a + 1) * P], pt[:])

                # phase-1's small inputs must hit the DMA engines BEFORE
                # the bulk ew prefetch: transfers run in issue order, so
                # issuing ew first would stall the gate ~35us.
                for a in range(EBLK):
                    xchunk_dma(a)
                for dsg in range(prefetch_dsgs):
                    issue_dsg(dsg)

                # constants (gpsimd iota + copies), after the DMA issues
                make_identity(nc, ident)
                nc.scalar.copy(ident16[:, :], ident)
                nc.vector.memset(sm16[:, 128:256], 1.0)

                # map extra x chunks (8..31) onto phase-2 dsg slots
                xtra = list(range(EBLK, TTILE))
                tdsg = {c: (i * NDSG) // len(xtra)
                        for i, c in enumerate(xtra)}
                dma_at = {}
                trans_at = {}
                for c in xtra:
                    dma_at.setdefault(max(0, tdsg[c] - 3), []).append(c)
                    trans_at.setdefault(tdsg[c], []).append(c)

                # ============ Phase 1: gate + softmax ==================
                with tc.tile_pool(name=f"dummyp{rep}", bufs=1,
                                  space="PSUM") as dummyp, \
                     tc.tile_pool(name=f"tpsum{rep}", bufs=2,
                                  space="PSUM") as tpsum, \
                     tc.tile_pool(name=f"gpsum{rep}", bufs=2,
                                  space="PSUM") as gpsum:

                    dummy = dummyp.tile([1, 1], f32)
                    # PE touch: absorb gpsimd tick (identity)
                    nc.tensor.matmul(dummy[:], ident16[:, 0:1],
                                     ident16[:, 0:1],
                                     start=True, stop=True)

                    for a in range(EBLK):
                        xchunk_transpose(a, tpsum, "pt")

                    for a in range(EBLK):
                        lp = gpsum.tile([P, F], f32, tag="lp",
                                        name=f"lp{rep}_{a}")
                        for half in range(2):
                            sl = slice(half * 512, (half + 1) * 512)
                            nc.tensor.matmul(lp[:, sl],
                                             xT[:, 0, a * P:(a + 1) * P],
                                             gw_sb[:, 0, sl],
                                             start=True, stop=False)
                            nc.tensor.matmul(lp[:, sl],
                                             xT[:, 1, a * P:(a + 1) * P],
                                             gw_sb[:, 1, sl],
                                             start=False, stop=False)
                            nc.tensor.matmul(lp[:, sl], ones16_r,
                                             gb_sb[0:1, sl],
                                             start=False, stop=True)
                        nc.scalar.activation(expsc[:], lp[:], AF.Exp,
                                             accum_out=rsum)
                        nc.vector.reciprocal(rcp, rsum)
                        nc.vector.tensor_scalar_mul(
                            wnorm16[:, a, :], expsc[:, 0:FSH], rcp)

                    # ====== b_eff via PE diag trick ====================
                    bp = tpsum.tile([P, P], f32, tag="bpt", bufs=1,
                                    name=f"bp{rep}")
                    for et in range(EBLK):
                        nc.tensor.matmul(bp[:], wnorm16[:, et, :],
                                         ebT_sb[:, et, :],
                                         start=(et == 0),
                                         stop=(et == EBLK - 1))
                    nc.vector.scalar_tensor_tensor(
                        out=junk[:], in0=bp[:], scalar=1.0,
                        in1=ident, op0=mult, op1=mult,
                        accum_out=beff_col)

                # == Phase 2: W_eff^T diag-matmul on PE, extract on DVE ==
                # x-chunk transposes for chunks 8..31 ride along here
                # (PE is ~40% idle while the DMA stream paces the phase).
                with tc.tile_pool(name=f"wpsum{rep}", bufs=6,
                                  space="PSUM") as wpsum:
                    for dsg in range(NDSG):
                        if dsg + prefetch_dsgs < NDSG:
                            issue_dsg(dsg + prefetch_dsgs)
                        for c in dma_at.get(dsg, []):
                            xchunk_dma(c)
                        for c in trans_at.get(dsg, []):
                            xchunk_transpose(c, wpsum, "pt2")
                        banks = []
                        for b in range(NBANK):
                            bk = wpsum.tile([P, 512], f32, tag="wp",
                                            name=f"wp{rep}_{dsg}_{b}")
                            banks.append(bk)
                        wets = wet_tiles.pop(dsg)
                        for b in range(NBANK):
                            for et in range(EBLK):
                                nc.tensor.matmul(
                                    banks[b][:],
                                    wnorm16[:, et, :],
                                    wets[et][:, b * 4:(b + 1) * 4, :],
                                    start=(et == 0),
                                    stop=(et == EBLK - 1))
                        for b in range(NBANK):
                            for di in range(4):
                                d = dsg * dsg_size + b * 4 + di
                                nc.vector.scalar_tensor_tensor(
                                    out=junk[:],
                                    in0=banks[b][:, di * P:(di + 1) * P],
                                    scalar=1.0, in1=ident,
                                    op0=mult, op1=mult,
                                    accum_out=wefft[:, d:d + 1])

                # ====== Phase 3: W_eff transpose + final GEMM ==========
                with tc.tile_pool(name=f"fpsum{rep}", bufs=2,
                                  space="PSUM") as fpsum:
                    for dh in range(DH):
                        pt3 = fpsum.tile([P, P], f32, tag="pt3",
                                         name=f"pt3{rep}_{dh}")
                        nc.tensor.transpose(
                            pt3[:], wefft[:, dh * P:(dh + 1) * P], ident)
                        nc.scalar.copy(
                            weff[:, dh * FSH:(dh + 1) * FSH], pt3[:])
                    for ch in range(N // 512):
                        sl = slice(ch * 512, (ch + 1) * 512)
                        ps = fpsum.tile([P, 512], f32, tag="fp",
                                        name=f"fp{rep}_{ch}")
                        nc.tensor.matmul(ps[:], weff[:, 0:FSH],
                                         xT[:, 0, sl],
                                         start=True, stop=False)
                        nc.tensor.matmul(ps[:], weff[:, FSH:2 * FSH],
                                         xT[:, 1, sl],
                                         start=False, stop=True)
                        # psum->SBUF copy with per-partition bias add
                        nc.scalar.activation(outT_sb[:, sl], ps[:],
                                             AF.Identity, bias=beff_col,
                                             scale=1.0)
                        nc.sync.dma_start(out=out_d[:, sl],
                                          in_=outT_sb[:, sl])

    _split_multi_waits(nc)
    return nc


def _prep_in_maps(x, gate_W, gate_b, expert_W, expert_b):
    x16 = np.ascontiguousarray(np.asarray(x).astype(np.float16))
    gate_W = np.asarray(gate_W, dtype=np.float32)
    gate_b = np.asarray(gate_b, dtype=np.float32).reshape(1, F)
    expert_W = np.asarray(expert_W, dtype=np.float32)
    expert_b = np.asarray(expert_b, dtype=np.float32)

    in_maps = []
    for c in range(NCORES):
        sh = slice(c * FSH, (c + 1) * FSH)
        in_maps.append({
            "x": x16,
            # roll shard columns to the front; softmax row-sum is invariant
            "gw": np.ascontiguousarray(
                np.roll(gate_W, -c * FSH, axis=1).astype(np.float16)),
            "gb": np.ascontiguousarray(
                np.roll(gate_b, -c * FSH, axis=1).astype(np.float16)),
            # natural [e, d, f] / [e, f] order, fp16
            "ewt": np.ascontiguousarray(
                expert_W[:, :, sh].astype(np.float16)),
            "ebt": np.ascontiguousarray(
                expert_b[:, sh].astype(np.float16)),
        })
    return in_maps


def kernel(x, gate_W, gate_b, expert_W, expert_b, _trace=False):
    global LAST_RESULT
    from concourse.bass_utils import run_bass_kernel_spmd

    if "nc" not in _CACHE:
        _CACHE["nc"] = _build_bass()
    nc = _CACHE["nc"]

    in_maps = _prep_in_maps(x, gate_W, gate_b, expert_W, expert_b)

    res = run_bass_kernel_spmd(
        nc, in_maps, core_ids=list(range(NCORES)), trace=_trace,
    )
    LAST_RESULT = res

    out = np.empty([N, F], dtype=np.float32)
    for c in range(NCORES):
        out[:, c * FSH:(c + 1) * FSH] = \
            res.results[c]["outT"].astype(np.float32).T
    return out
